# revision 36
# baseline (speedup 1.0000x reference)
"""Trainium2 Bass kernel for the gated-cell module:

    rt = sigmoid(xt @ Wa.T + ba); it = sigmoid(xt @ Wx.T + bx)
    at = exp(-(C*softplus(Lambda)) * rt)
    ht = at * ht_minus_1 + sqrt(1 - at^2) * (it * xt)

Sharding: data-parallel over the batch dim across 8 NeuronCores; weights
replicated.  Compute runs in a transposed layout ([D, B] with D on the
partition axis) so the per-feature vectors (ba, bx, -C*softplus(Lambda))
ride in the ACT engine's per-partition scale/bias operands, and xt is
already K-major for the PE.

Matmuls and element-wise intermediates run in bf16 (fp32 PSUM
accumulation, fp32 output): bf16 matmul streams at full PE rate while
fp32 runs at 1/4, and bf16 doubles DVE throughput.  sqrt(1-at^2) is
computed as exp(0.5*ln(1-at^2)) on wide group tiles; ACT instructions
are chained in emission order (sync=False deps) so the scheduler cannot
interleave different ACT table sets — every alternation would cost a
~1.5us ACT_TABLE_LOAD.
"""

import sys

if "/opt/trn_rl_repo" not in sys.path:
    sys.path.insert(0, "/opt/trn_rl_repo")

import numpy as np

B, D = 16384, 1024
C = 8.0
NCORES = 8
BS = B // NCORES          # 2048 batch rows per core
PT = 128                  # partition tile
KT = D // PT              # 8 k-tiles (contraction)
JT = D // PT              # 8 j-tiles (output features)
CHUNKS = (512, 512, 512, 512)  # batch-chunk widths per core (== BS)
# last chunk is small to shrink the post-matmul phase-2 tail

_CACHE = {}


def _build(fused_tanh=True):
    """fused_tanh: single tanh per j over a combined [za|zx] PSUM tile.
    Valid only when ba == bx == 0 (true for this problem's inputs);
    _prep() falls back to the general build otherwise."""
    from contextlib import ExitStack

    import concourse.mybir as mybir
    import concourse.tile as tile
    from concourse.tile import add_dep_helper
    from concourse import bacc

    f32 = mybir.dt.float32
    bf16 = mybir.dt.bfloat16
    AF = mybir.ActivationFunctionType
    ALU = mybir.AluOpType

    nc = bacc.Bacc("TRN2", target_bir_lowering=False, debug=False,
                   num_devices=NCORES, dynamic_dma_scratch_size=4096)

    xtT = nc.dram_tensor("xtT", [D, BS], bf16, kind="ExternalInput").ap()
    htT = nc.dram_tensor("htT", [D, BS], bf16, kind="ExternalInput").ap()
    waT = nc.dram_tensor("waT", [D, D], bf16, kind="ExternalInput").ap()
    wxT = nc.dram_tensor("wxT", [D, D], bf16, kind="ExternalInput").ap()
    # consts[:, 0:JT] = ba/2, [:, JT:2JT] = bx/2, [:, 2JT:3JT] = negk/2,
    # [:, 3JT] = ln(0.5)
    consts = nc.dram_tensor("consts", [PT, 3 * JT + 1], f32,
                            kind="ExternalInput").ap()
    outT = nc.dram_tensor("outT", [D, BS], f32, kind="ExternalOutput").ap()

    ln_ops = [[] for _ in CHUNKS]  # natural-log-set ACT ops per chunk

    with tile.TileContext(nc) as tc, ExitStack() as ctx:
        wpool = ctx.enter_context(tc.tile_pool(name="w", bufs=1))
        cpool = ctx.enter_context(tc.tile_pool(name="c", bufs=1))
        xpool = ctx.enter_context(tc.tile_pool(name="x", bufs=2))
        gpool = ctx.enter_context(tc.tile_pool(name="g", bufs=2))
        tpool = ctx.enter_context(tc.tile_pool(name="t", bufs=1))
        pzpool = ctx.enter_context(tc.tile_pool(name="pz", bufs=2, space="PSUM"))

        # DMA order: chunk-0 x, Wa, consts, Wx — the first accumulation
        # group starts after x+Wa; biases are first needed at tanh time.
        Q0 = CHUNKS[0]
        x_g0 = xpool.tile([PT, KT, Q0], bf16, name="xg0", tag="x")
        wa_g = wpool.tile([PT, KT, D], bf16, name="wag", tag="wa")
        # Two interleaved halves: the k<4 matmuls can start after ~1.5MB.
        for kh in (slice(0, KT // 2), slice(KT // 2, KT)):
            nc.sync.dma_start(
                out=x_g0[:, kh, :],
                in_=xtT[kh.start * PT:kh.stop * PT, 0:Q0].rearrange(
                    "(kt p) q -> p kt q", p=PT))
            nc.sync.dma_start(
                out=wa_g[:, kh, :],
                in_=waT[kh.start * PT:kh.stop * PT, :].rearrange(
                    "(kt p) j -> p kt j", p=PT))
        c_sb = cpool.tile([PT, 3 * JT + 1], f32, tag="c")
        nc.sync.dma_start(out=c_sb, in_=consts)
        ba2_sb = c_sb[:, 0:JT]
        bx2_sb = c_sb[:, JT:2 * JT]
        nk2_sb = c_sb[:, 2 * JT:3 * JT]
        lnhalf_sb = c_sb[:, 3 * JT:3 * JT + 1]
        wx_g = wpool.tile([PT, KT, D], bf16, name="wxg", tag="wx")
        for kh in (slice(0, KT // 2), slice(KT // 2, KT)):
            nc.sync.dma_start(
                out=wx_g[:, kh, :],
                in_=wxT[kh.start * PT:kh.stop * PT, :].rearrange(
                    "(kt p) j -> p kt j", p=PT))

        coff = 0
        x_g = x_g0
        for ci, Q in enumerate(CHUNKS):
            bsl = slice(coff, coff + Q)
            coff += Q
            nsls = []
            off = 0
            while off < Q:
                w = min(512, Q - off)
                nsls.append(slice(off, off + w))
                off += w

            # rp_g[:, j, 0, :] = Ta (-> at), rp_g[:, j, 1, :] = Tx (-> p')
            rp_g = gpool.tile([PT, JT, 2, Q], bf16, tag="rp", name=f"rp{ci}")

            # ---- phase 1: GEMMs (bf16, fp32 PSUM); one tanh per j ----
            # rt = 0.5 + 0.5*tanh(za/2); tanh shares its ACT table set with
            # exp, so phase 1 and phase 2 never thrash table sets.
            for j in range(JT):
                jsl = slice(j * PT, (j + 1) * PT)
                if fused_tanh:
                    # [za | zx] in one 4-bank tile; each 1024-wide half is
                    # bank-aligned so every matmul piece stays in one bank.
                    zz = pzpool.tile([PT, 2, 512], f32, tag="zz",
                                     name=f"zz{ci}_{j}", bufs=4)
                    za = zz[:, 0, 0:Q]
                    zx = zz[:, 1, 0:Q]
                else:
                    za = pzpool.tile([PT, Q], f32, tag="za",
                                     name=f"za{ci}_{j}")
                    zx = pzpool.tile([PT, Q], f32, tag="zx",
                                     name=f"zx{ci}_{j}")
                for k in range(KT):
                    for nsl in nsls:
                        nc.tensor.matmul(za[:, nsl], wa_g[:, k, jsl],
                                         x_g[:, k, nsl],
                                         start=(k == 0), stop=(k == KT - 1))
                for k in range(KT):
                    for nsl in nsls:
                        nc.tensor.matmul(zx[:, nsl], wx_g[:, k, jsl],
                                         x_g[:, k, nsl],
                                         start=(k == 0), stop=(k == KT - 1))
                if fused_tanh:
                    nc.scalar.activation(out=rp_g[:, j, :, :],
                                         in_=zz[:, :, 0:Q],
                                         func=AF.Tanh, scale=0.5)
                else:
                    nc.scalar.activation(out=rp_g[:, j, 0, :], in_=za,
                                         func=AF.Tanh,
                                         bias=ba2_sb[:, j:j + 1], scale=0.5)
                    nc.scalar.activation(out=rp_g[:, j, 1, :], in_=zx,
                                         func=AF.Tanh,
                                         bias=bx2_sb[:, j:j + 1], scale=0.5)
                # p' = (Tx + 1) * x  (= 2*it*xt; the 1/2 folds into s')
                nc.vector.scalar_tensor_tensor(
                    out=rp_g[:, j, 1, :], in0=rp_g[:, j, 1, :], scalar=1.0,
                    in1=x_g[:, j, :], op0=ALU.add, op1=ALU.mult)

            # Prefetch next chunk's x before this chunk's h/out DMAs enter
            # the sync queue (HWDGE is FIFO per engine).
            if ci + 1 < len(CHUNKS):
                Qn = CHUNKS[ci + 1]
                nbsl = slice(coff, coff + Qn)
                x_g = xpool.tile([PT, KT, Qn], bf16, name=f"xg{ci+1}",
                                 tag="x")
                nc.sync.dma_start(
                    out=x_g,
                    in_=xtT[:, nbsl].rearrange("(kt p) q -> p kt q", p=PT))

            # ---- phase 2 ----
            # Ta <- negk/2 * Ta + negk/2 (== negk*rt), then exp -> at
            for j in range(JT):
                nc.vector.tensor_scalar(
                    out=rp_g[:, j, 0, :], in0=rp_g[:, j, 0, :],
                    scalar1=nk2_sb[:, j:j + 1], scalar2=nk2_sb[:, j:j + 1],
                    op0=ALU.mult, op1=ALU.add)

            h_g = tpool.tile([PT, JT, Q], bf16, tag="h", name=f"h{ci}")
            nwave = 4 if ci == len(CHUNKS) - 1 else 2
            wj = JT // nwave
            halves = [slice(i * wj, (i + 1) * wj) for i in range(nwave)]
            for hs in halves:
                nc.sync.dma_start(
                    out=h_g[:, hs, :],
                    in_=htT[hs.start * PT:hs.stop * PT, bsl].rearrange(
                        "(jt p) q -> p jt q", p=PT))

            a2_g = gpool.tile([PT, JT, Q], bf16, tag="a2", name=f"a2{ci}")
            m1_g = gpool.tile([PT, JT, Q], bf16, tag="m1", bufs=1,
                              name=f"m1{ci}")
            o_g = gpool.tile([PT, JT, Q], f32, tag="o", bufs=1, name=f"o{ci}")

            for hs in halves:
                nc.scalar.activation(out=rp_g[:, hs, 0, :],
                                     in_=rp_g[:, hs, 0, :], func=AF.Exp)
            for j in range(JT):
                nc.vector.tensor_mul(a2_g[:, j, :], rp_g[:, j, 0, :],
                                     rp_g[:, j, 0, :])
                nc.vector.tensor_mul(m1_g[:, j, :], rp_g[:, j, 0, :],
                                     h_g[:, j, :])
            # One atomic full-width Ln per chunk: an ordering dep alone
            # cannot stop a (same-set-free) tanh from landing between two
            # Ln halves, which would cost two extra table loads.  The last
            # chunk has no tanhs after it, so there it is safe to split the
            # Ln into waves, shortening the end-of-kernel serial chain.
            if ci == len(CHUNKS) - 1:
                for hs in halves:
                    ln_ops[ci].append(
                        nc.scalar.activation(out=a2_g[:, hs, :],
                                             in_=a2_g[:, hs, :], func=AF.Ln,
                                             bias=1.0, scale=-1.0))
            else:
                ln_ops[ci].append(
                    nc.scalar.activation(out=a2_g, in_=a2_g, func=AF.Ln,
                                         bias=1.0, scale=-1.0))
            for hs in halves:
                # s' = exp(0.5*ln(1-at^2) + ln(0.5)) = sqrt(1-at^2)/2
                nc.scalar.activation(out=a2_g[:, hs, :], in_=a2_g[:, hs, :],
                                     func=AF.Exp, scale=0.5, bias=lnhalf_sb)
            for hs in halves:
                for j in range(hs.start, hs.stop):
                    nc.vector.tensor_mul(rp_g[:, j, 1, :], a2_g[:, j, :],
                                         rp_g[:, j, 1, :])
                    nc.vector.tensor_add(o_g[:, j, :], m1_g[:, j, :],
                                         rp_g[:, j, 1, :])
                nc.sync.dma_start(
                    out=outT[hs.start * PT:hs.stop * PT, bsl].rearrange(
                        "(jt p) q -> p jt q", p=PT),
                    in_=o_g[:, hs, :])

        # Keep a chunk's Ln ops adjacent on the ACT stream so the
        # natural-log table set loads once per chunk.
        for ops in ln_ops:
            for a, b in zip(ops, ops[1:]):
                add_dep_helper(b.ins, a.ins, sync=False, reason="ln adjacency")

    nc.compile()
    return nc


def _np_softplus(x):
    return np.logaddexp(0.0, x)


def _fold(vec):
    # [D] feature vector -> [128, JT] tile where column j holds features
    # j*128 .. j*128+127 (per-partition scalars for j-tile j).
    return np.ascontiguousarray(vec.reshape(JT, PT).T)


def _prep(xt, ht, Wa, Wx, ba, bx, Lam):
    import ml_dtypes

    bf16 = ml_dtypes.bfloat16
    negk_vec = (-C * _np_softplus(Lam.astype(np.float64))).astype(np.float32)
    xtT = np.ascontiguousarray(xt.T.astype(bf16))
    htT = np.ascontiguousarray(ht.T.astype(bf16))
    waT = np.ascontiguousarray(Wa.T.astype(bf16))
    wxT = np.ascontiguousarray(Wx.T.astype(bf16))
    consts = np.concatenate(
        [_fold(0.5 * ba), _fold(0.5 * bx), _fold(0.5 * negk_vec),
         np.full((PT, 1), np.log(0.5), np.float32)], axis=1)
    consts = np.ascontiguousarray(consts)
    in_maps = []
    for c in range(NCORES):
        sl = slice(c * BS, (c + 1) * BS)
        in_maps.append({
            "xtT": np.ascontiguousarray(xtT[:, sl]),
            "htT": np.ascontiguousarray(htT[:, sl]),
            "waT": waT,
            "wxT": wxT,
            "consts": consts,
        })
    return in_maps


def kernel(xt, ht_minus_1, Wa, Wx, ba, bx, Lambda):
    from concourse.bass_utils import run_bass_kernel_spmd

    fused = (not np.any(np.asarray(ba))) and (not np.any(np.asarray(bx)))
    key = ("nc", fused)
    if key not in _CACHE:
        _CACHE[key] = _build(fused_tanh=fused)
    nc = _CACHE[key]

    in_maps = _prep(
        np.asarray(xt, dtype=np.float32),
        np.asarray(ht_minus_1, dtype=np.float32),
        np.asarray(Wa, dtype=np.float32),
        np.asarray(Wx, dtype=np.float32),
        np.asarray(ba, dtype=np.float32).reshape(-1),
        np.asarray(bx, dtype=np.float32).reshape(-1),
        np.asarray(Lambda, dtype=np.float32).reshape(-1),
    )
    res = run_bass_kernel_spmd(nc, in_maps, list(range(NCORES)))
    outT = np.concatenate([res.results[c]["outT"] for c in range(NCORES)],
                          axis=1)
    return np.ascontiguousarray(outT.T)


# revision 37
# speedup vs baseline: 1.0049x; 1.0049x over previous
"""Trainium2 Bass kernel for the gated-cell module:

    rt = sigmoid(xt @ Wa.T + ba); it = sigmoid(xt @ Wx.T + bx)
    at = exp(-(C*softplus(Lambda)) * rt)
    ht = at * ht_minus_1 + sqrt(1 - at^2) * (it * xt)

Sharding: data-parallel over the batch dim across 8 NeuronCores; weights
replicated.  Compute runs in a transposed layout ([D, B] with D on the
partition axis) so the per-feature vectors (ba, bx, -C*softplus(Lambda))
ride in the ACT engine's per-partition scale/bias operands, and xt is
already K-major for the PE.

Matmuls and element-wise intermediates run in bf16 (fp32 PSUM
accumulation, fp32 output): bf16 matmul streams at full PE rate while
fp32 runs at 1/4, and bf16 doubles DVE throughput.  sqrt(1-at^2) is
computed as exp(0.5*ln(1-at^2)) on wide group tiles; ACT instructions
are chained in emission order (sync=False deps) so the scheduler cannot
interleave different ACT table sets — every alternation would cost a
~1.5us ACT_TABLE_LOAD.
"""

import sys

if "/opt/trn_rl_repo" not in sys.path:
    sys.path.insert(0, "/opt/trn_rl_repo")

import numpy as np

B, D = 16384, 1024
C = 8.0
NCORES = 8
BS = B // NCORES          # 2048 batch rows per core
PT = 128                  # partition tile
KT = D // PT              # 8 k-tiles (contraction)
JT = D // PT              # 8 j-tiles (output features)
CHUNKS = (512, 512, 512, 512)  # batch-chunk widths per core (== BS)
# last chunk is small to shrink the post-matmul phase-2 tail

_CACHE = {}


def _build(fused_tanh=True):
    """fused_tanh: single tanh per j over a combined [za|zx] PSUM tile.
    Valid only when ba == bx == 0 (true for this problem's inputs);
    _prep() falls back to the general build otherwise."""
    from contextlib import ExitStack

    import concourse.mybir as mybir
    import concourse.tile as tile
    from concourse.tile import add_dep_helper
    from concourse import bacc

    f32 = mybir.dt.float32
    bf16 = mybir.dt.bfloat16
    AF = mybir.ActivationFunctionType
    ALU = mybir.AluOpType

    nc = bacc.Bacc("TRN2", target_bir_lowering=False, debug=False,
                   num_devices=NCORES, dynamic_dma_scratch_size=4096)

    xtT = nc.dram_tensor("xtT", [D, BS], bf16, kind="ExternalInput").ap()
    htT = nc.dram_tensor("htT", [D, BS], bf16, kind="ExternalInput").ap()
    waT = nc.dram_tensor("waT", [D, D], bf16, kind="ExternalInput").ap()
    wxT = nc.dram_tensor("wxT", [D, D], bf16, kind="ExternalInput").ap()
    # consts[:, 0:JT] = ba/2, [:, JT:2JT] = bx/2, [:, 2JT:3JT] = negk/2,
    # [:, 3JT] = ln(0.5)
    consts = nc.dram_tensor("consts", [PT, 3 * JT + 1], f32,
                            kind="ExternalInput").ap()
    outT = nc.dram_tensor("outT", [D, BS], f32, kind="ExternalOutput").ap()

    ln_ops = [[] for _ in CHUNKS]  # natural-log-set ACT ops per chunk

    with tile.TileContext(nc) as tc, ExitStack() as ctx:
        wpool = ctx.enter_context(tc.tile_pool(name="w", bufs=1))
        cpool = ctx.enter_context(tc.tile_pool(name="c", bufs=1))
        xpool = ctx.enter_context(tc.tile_pool(name="x", bufs=2))
        gpool = ctx.enter_context(tc.tile_pool(name="g", bufs=2))
        tpool = ctx.enter_context(tc.tile_pool(name="t", bufs=1))
        pzpool = ctx.enter_context(tc.tile_pool(name="pz", bufs=2, space="PSUM"))

        # DMA order: chunk-0 x, Wa, consts, Wx — the first accumulation
        # group starts after x+Wa; biases are first needed at tanh time.
        Q0 = CHUNKS[0]
        x_g0 = xpool.tile([PT, KT, Q0], bf16, name="xg0", tag="x")
        wa_g = wpool.tile([PT, KT, D], bf16, name="wag", tag="wa")
        # Two interleaved halves: the k<4 matmuls can start after ~1.5MB.
        for kh in (slice(0, KT // 2), slice(KT // 2, KT)):
            nc.sync.dma_start(
                out=x_g0[:, kh, :],
                in_=xtT[kh.start * PT:kh.stop * PT, 0:Q0].rearrange(
                    "(kt p) q -> p kt q", p=PT))
            nc.sync.dma_start(
                out=wa_g[:, kh, :],
                in_=waT[kh.start * PT:kh.stop * PT, :].rearrange(
                    "(kt p) j -> p kt j", p=PT))
        c_sb = cpool.tile([PT, 3 * JT + 1], f32, tag="c")
        nc.sync.dma_start(out=c_sb, in_=consts)
        ba2_sb = c_sb[:, 0:JT]
        bx2_sb = c_sb[:, JT:2 * JT]
        nk2_sb = c_sb[:, 2 * JT:3 * JT]
        lnhalf_sb = c_sb[:, 3 * JT:3 * JT + 1]
        wx_g = wpool.tile([PT, KT, D], bf16, name="wxg", tag="wx")
        for kh in (slice(0, KT // 2), slice(KT // 2, KT)):
            nc.sync.dma_start(
                out=wx_g[:, kh, :],
                in_=wxT[kh.start * PT:kh.stop * PT, :].rearrange(
                    "(kt p) j -> p kt j", p=PT))

        coff = 0
        x_g = x_g0
        for ci, Q in enumerate(CHUNKS):
            bsl = slice(coff, coff + Q)
            coff += Q
            nsls = []
            off = 0
            while off < Q:
                w = min(512, Q - off)
                nsls.append(slice(off, off + w))
                off += w

            # rp_g[:, j, 0, :] = Ta (-> at), rp_g[:, j, 1, :] = Tx (-> p')
            rp_g = gpool.tile([PT, JT, 2, Q], bf16, tag="rp", name=f"rp{ci}")

            # ---- phase 1: GEMMs (bf16, fp32 PSUM); one tanh per j ----
            # rt = 0.5 + 0.5*tanh(za/2); tanh shares its ACT table set with
            # exp, so phase 1 and phase 2 never thrash table sets.
            for j in range(JT):
                jsl = slice(j * PT, (j + 1) * PT)
                if fused_tanh:
                    # [za | zx] in one 4-bank tile; each 1024-wide half is
                    # bank-aligned so every matmul piece stays in one bank.
                    zz = pzpool.tile([PT, 2, 512], f32, tag="zz",
                                     name=f"zz{ci}_{j}", bufs=4)
                    za = zz[:, 0, 0:Q]
                    zx = zz[:, 1, 0:Q]
                else:
                    za = pzpool.tile([PT, Q], f32, tag="za",
                                     name=f"za{ci}_{j}")
                    zx = pzpool.tile([PT, Q], f32, tag="zx",
                                     name=f"zx{ci}_{j}")
                for k in range(KT):
                    for nsl in nsls:
                        nc.tensor.matmul(za[:, nsl], wa_g[:, k, jsl],
                                         x_g[:, k, nsl],
                                         start=(k == 0), stop=(k == KT - 1))
                for k in range(KT):
                    for nsl in nsls:
                        nc.tensor.matmul(zx[:, nsl], wx_g[:, k, jsl],
                                         x_g[:, k, nsl],
                                         start=(k == 0), stop=(k == KT - 1))
                if fused_tanh:
                    nc.scalar.activation(out=rp_g[:, j, :, :],
                                         in_=zz[:, :, 0:Q],
                                         func=AF.Tanh, scale=0.5)
                else:
                    nc.scalar.activation(out=rp_g[:, j, 0, :], in_=za,
                                         func=AF.Tanh,
                                         bias=ba2_sb[:, j:j + 1], scale=0.5)
                    nc.scalar.activation(out=rp_g[:, j, 1, :], in_=zx,
                                         func=AF.Tanh,
                                         bias=bx2_sb[:, j:j + 1], scale=0.5)
                # p' = (Tx + 1) * x  (= 2*it*xt; the 1/2 folds into s')
                nc.vector.scalar_tensor_tensor(
                    out=rp_g[:, j, 1, :], in0=rp_g[:, j, 1, :], scalar=1.0,
                    in1=x_g[:, j, :], op0=ALU.add, op1=ALU.mult)

            # Prefetch next chunk's x before this chunk's h/out DMAs enter
            # the sync queue (HWDGE is FIFO per engine).
            if ci + 1 < len(CHUNKS):
                Qn = CHUNKS[ci + 1]
                nbsl = slice(coff, coff + Qn)
                x_g = xpool.tile([PT, KT, Qn], bf16, name=f"xg{ci+1}",
                                 tag="x")
                for kh in (slice(0, KT // 2), slice(KT // 2, KT)):
                    nc.sync.dma_start(
                        out=x_g[:, kh, :],
                        in_=xtT[kh.start * PT:kh.stop * PT, nbsl].rearrange(
                            "(kt p) q -> p kt q", p=PT))

            # ---- phase 2 ----
            # Ta <- negk/2 * Ta + negk/2 (== negk*rt), then exp -> at
            for j in range(JT):
                nc.vector.tensor_scalar(
                    out=rp_g[:, j, 0, :], in0=rp_g[:, j, 0, :],
                    scalar1=nk2_sb[:, j:j + 1], scalar2=nk2_sb[:, j:j + 1],
                    op0=ALU.mult, op1=ALU.add)

            h_g = tpool.tile([PT, JT, Q], bf16, tag="h", name=f"h{ci}")
            nwave = 4 if ci == len(CHUNKS) - 1 else 2
            wj = JT // nwave
            halves = [slice(i * wj, (i + 1) * wj) for i in range(nwave)]
            for hs in halves:
                nc.sync.dma_start(
                    out=h_g[:, hs, :],
                    in_=htT[hs.start * PT:hs.stop * PT, bsl].rearrange(
                        "(jt p) q -> p jt q", p=PT))

            a2_g = gpool.tile([PT, JT, Q], bf16, tag="a2", name=f"a2{ci}")
            m1_g = gpool.tile([PT, JT, Q], bf16, tag="m1", bufs=1,
                              name=f"m1{ci}")
            o_g = gpool.tile([PT, JT, Q], f32, tag="o", bufs=1, name=f"o{ci}")

            for hs in halves:
                nc.scalar.activation(out=rp_g[:, hs, 0, :],
                                     in_=rp_g[:, hs, 0, :], func=AF.Exp)
            for j in range(JT):
                nc.vector.tensor_mul(a2_g[:, j, :], rp_g[:, j, 0, :],
                                     rp_g[:, j, 0, :])
                nc.vector.tensor_mul(m1_g[:, j, :], rp_g[:, j, 0, :],
                                     h_g[:, j, :])
            # One atomic full-width Ln per chunk: an ordering dep alone
            # cannot stop a (same-set-free) tanh from landing between two
            # Ln halves, which would cost two extra table loads.  The last
            # chunk has no tanhs after it, so there it is safe to split the
            # Ln into waves, shortening the end-of-kernel serial chain.
            if ci == len(CHUNKS) - 1:
                for hs in halves:
                    ln_ops[ci].append(
                        nc.scalar.activation(out=a2_g[:, hs, :],
                                             in_=a2_g[:, hs, :], func=AF.Ln,
                                             bias=1.0, scale=-1.0))
            else:
                ln_ops[ci].append(
                    nc.scalar.activation(out=a2_g, in_=a2_g, func=AF.Ln,
                                         bias=1.0, scale=-1.0))
            for hs in halves:
                # s' = exp(0.5*ln(1-at^2) + ln(0.5)) = sqrt(1-at^2)/2
                nc.scalar.activation(out=a2_g[:, hs, :], in_=a2_g[:, hs, :],
                                     func=AF.Exp, scale=0.5, bias=lnhalf_sb)
            for hs in halves:
                for j in range(hs.start, hs.stop):
                    nc.vector.tensor_mul(rp_g[:, j, 1, :], a2_g[:, j, :],
                                         rp_g[:, j, 1, :])
                    nc.vector.tensor_add(o_g[:, j, :], m1_g[:, j, :],
                                         rp_g[:, j, 1, :])
                nc.sync.dma_start(
                    out=outT[hs.start * PT:hs.stop * PT, bsl].rearrange(
                        "(jt p) q -> p jt q", p=PT),
                    in_=o_g[:, hs, :])

        # Keep a chunk's Ln ops adjacent on the ACT stream so the
        # natural-log table set loads once per chunk.
        for ops in ln_ops:
            for a, b in zip(ops, ops[1:]):
                add_dep_helper(b.ins, a.ins, sync=False, reason="ln adjacency")

    nc.compile()
    return nc


def _np_softplus(x):
    return np.logaddexp(0.0, x)


def _fold(vec):
    # [D] feature vector -> [128, JT] tile where column j holds features
    # j*128 .. j*128+127 (per-partition scalars for j-tile j).
    return np.ascontiguousarray(vec.reshape(JT, PT).T)


def _prep(xt, ht, Wa, Wx, ba, bx, Lam):
    import ml_dtypes

    bf16 = ml_dtypes.bfloat16
    negk_vec = (-C * _np_softplus(Lam.astype(np.float64))).astype(np.float32)
    xtT = np.ascontiguousarray(xt.T.astype(bf16))
    htT = np.ascontiguousarray(ht.T.astype(bf16))
    waT = np.ascontiguousarray(Wa.T.astype(bf16))
    wxT = np.ascontiguousarray(Wx.T.astype(bf16))
    consts = np.concatenate(
        [_fold(0.5 * ba), _fold(0.5 * bx), _fold(0.5 * negk_vec),
         np.full((PT, 1), np.log(0.5), np.float32)], axis=1)
    consts = np.ascontiguousarray(consts)
    in_maps = []
    for c in range(NCORES):
        sl = slice(c * BS, (c + 1) * BS)
        in_maps.append({
            "xtT": np.ascontiguousarray(xtT[:, sl]),
            "htT": np.ascontiguousarray(htT[:, sl]),
            "waT": waT,
            "wxT": wxT,
            "consts": consts,
        })
    return in_maps


def kernel(xt, ht_minus_1, Wa, Wx, ba, bx, Lambda):
    from concourse.bass_utils import run_bass_kernel_spmd

    fused = (not np.any(np.asarray(ba))) and (not np.any(np.asarray(bx)))
    key = ("nc", fused)
    if key not in _CACHE:
        _CACHE[key] = _build(fused_tanh=fused)
    nc = _CACHE[key]

    in_maps = _prep(
        np.asarray(xt, dtype=np.float32),
        np.asarray(ht_minus_1, dtype=np.float32),
        np.asarray(Wa, dtype=np.float32),
        np.asarray(Wx, dtype=np.float32),
        np.asarray(ba, dtype=np.float32).reshape(-1),
        np.asarray(bx, dtype=np.float32).reshape(-1),
        np.asarray(Lambda, dtype=np.float32).reshape(-1),
    )
    res = run_bass_kernel_spmd(nc, in_maps, list(range(NCORES)))
    outT = np.concatenate([res.results[c]["outT"] for c in range(NCORES)],
                          axis=1)
    return np.ascontiguousarray(outT.T)
